# revision 26
# baseline (speedup 1.0000x reference)
"""Trainium2 Bass kernel for a single attention layer (Baichuan-style W_pack
attention with rotary embeddings), sharded over 8 NeuronCores:
tensor-parallel over 4 head groups x data-parallel over 2 batches.

Contract: kernel(**inputs) takes the FULL unsharded inputs and returns the
FULL output [2, 2048, 4096] float32. All sharding / gathering happens here.
"""

import math
import sys

import numpy as np

for _p in ("/opt/trn_rl_repo", "/root/.axon_site/_ro/trn_rl_repo"):
    if _p not in sys.path:
        sys.path.insert(0, _p)

HIDDEN = 4096
N_HEADS = 32
HEAD_DIM = 128
MAX_POS = 4096
BASE = 10000.0
B = 2
S = 2048
HEADS_PER_CORE = 8          # 32 heads / 4 groups
HG = 1024                   # head-group width = 8 heads * 128
NEG_BIG = -1.0e9

# RoPE partner permutation: quadrant q holds [lo_d 16q..16q+15, hi_d 64+16q..]
# so the rotate-half partner of new-row i is i+-16 inside its 32-row quadrant,
# reachable by DVE stream_shuffle.
PERM = np.zeros(128, dtype=np.int64)
for _q in range(4):
    PERM[32 * _q: 32 * _q + 16] = np.arange(16 * _q, 16 * _q + 16)
    PERM[32 * _q + 16: 32 * _q + 32] = 64 + np.arange(16 * _q, 16 * _q + 16)
SHUF_MASK = [(i + 16) % 32 for i in range(32)]
# sign of the sin term per (new) row: -1 where original d < 64
SIGN = np.where(PERM < 64, -1.0, 1.0).astype(np.float32)


def _rope_tables(max_pos):
    inv_freq = 1.0 / (BASE ** (np.arange(0, HEAD_DIM, 2, dtype=np.float32) / HEAD_DIM))
    t = np.arange(max_pos, dtype=np.float32)
    freqs = np.outer(t, inv_freq)                      # [P, 64]
    emb = np.concatenate((freqs, freqs), axis=-1)      # [P, 128]
    return np.cos(emb).astype(np.float32), np.sin(emb).astype(np.float32)


def _build_program(mask_mode):
    """mask_mode: 'causal' (block-skip + 4 triangle masks),
    'none' (dense, no mask), 'full' (dense, stream mask tiles)."""
    import concourse.bacc as bacc
    import concourse.mybir as mybir
    import concourse.tile as tile
    from contextlib import ExitStack

    F32 = mybir.dt.float32
    F32R = mybir.dt.float32r
    ALU = mybir.AluOpType
    ACTF = mybir.ActivationFunctionType

    nc = bacc.Bacc("TRN2", target_bir_lowering=False, debug=False)

    xT = nc.declare_dram_parameter("xT", [HIDDEN, S], F32, isOutput=False)
    wqkT = nc.declare_dram_parameter("wqkT", [HIDDEN, 2 * HG], F32, isOutput=False)
    wvT = nc.declare_dram_parameter("wvT", [HIDDEN, HG], F32, isOutput=False)
    woT = nc.declare_dram_parameter("woT", [HG, HIDDEN], F32, isOutput=False)
    cosT = nc.declare_dram_parameter("cosT", [128, S], F32, isOutput=False)
    sinT = nc.declare_dram_parameter("sinT", [128, S], F32, isOutput=False)
    if mask_mode == "causal":
        masks = nc.declare_dram_parameter("masks", [4, 128, 512], F32, isOutput=False)
    elif mask_mode == "full":
        maskT = nc.declare_dram_parameter("maskT", [S, S], F32, isOutput=False)
    out_p = nc.declare_dram_parameter("out_p", [S, HIDDEN], F32, isOutput=True)

    qT_s = nc.dram_tensor("qT_scratch", [HG, S], F32R)
    kT_s = nc.dram_tensor("kT_scratch", [HG, S], F32R)
    v_s = nc.dram_tensor("v_scratch", [S, HG], F32R)
    at_s = nc.dram_tensor("at_scratch", [HG, S], F32R)

    inv_sqrt_d = 1.0 / math.sqrt(HEAD_DIM)

    with tile.TileContext(nc, pool_alloc_mode="queue") as tc, ExitStack() as top:
        const_pool = top.enter_context(tc.tile_pool(name="consts", bufs=1))
        ones_f32 = const_pool.tile([128, 1], F32)
        nc.vector.memset(ones_f32, 1.0)
        ones_col = const_pool.tile([128, 1], F32R)
        nc.vector.tensor_copy(ones_col, ones_f32)
        ones_row_f32 = const_pool.tile([1, 128], F32)
        nc.vector.memset(ones_row_f32, 1.0)
        ones_row = const_pool.tile([1, 128], F32R)
        nc.vector.tensor_copy(ones_row, ones_row_f32)
        # ---------------- Phase A: QKV projection (+RoPE on q,k) -------------
        def emit_proj_half(hs):
            s0 = hs * 1024
            # h-chunk sweep order: B-block (16..31) first, A-block (0..15)
            # last, so the A pool's next-half refill overlaps B-block compute
            H_ORDER = list(range(16, 32)) + list(range(16))
            with ExitStack() as pha:
                xpoolA = pha.enter_context(tc.tile_pool(name="xhalfA", bufs=1))
                xpoolB = pha.enter_context(tc.tile_pool(name="xhalfB", bufs=1))
                xtA = xpoolA.tile([128, 16, 1024], F32R, name=f"xtA{hs}")
                xtB = xpoolB.tile([128, 16, 1024], F32R, name=f"xtB{hs}")
                xin = xT.ap()[:, s0:s0 + 1024].rearrange(
                    "(c p) s -> p c s", p=128).bitcast(F32R)
                for c in range(16, 32):
                    nc.sync.dma_start(out=xtB[:, c - 16, :], in_=xin[:, c, :])
                for c in range(16):
                    nc.sync.dma_start(out=xtA[:, c, :], in_=xin[:, c, :])

                def xt_slice(c, sl):
                    return xtB[:, c - 16, sl] if c >= 16 else xtA[:, c, sl]

                cspool = pha.enter_context(tc.tile_pool(name="cossin", bufs=1))
                cos_sb = cspool.tile([128, 1024], F32)
                nc.sync.dma_start(out=cos_sb, in_=cosT.ap()[:, s0:s0 + 1024])
                sin_sb = cspool.tile([128, 1024], F32)
                nc.sync.dma_start(out=sin_sb, in_=sinT.ap()[:, s0:s0 + 1024])

                # --- q,k projection, weight-stationary, out = projT [o, s] ---
                with ExitStack() as qk:
                    wpool = qk.enter_context(tc.tile_pool(name="wqk", bufs=2))
                    pqk = qk.enter_context(
                        tc.tile_pool(name="pqk", bufs=2, space="PSUM"))
                    rpool = qk.enter_context(tc.tile_pool(name="rope", bufs=2))
                    spill = qk.enter_context(tc.tile_pool(name="spillqk", bufs=3))
                    for oc in range(16):         # o chunks of 128 (head tiles)
                        w_oc = wpool.tile([128, 32, 128], F32R, tag="w_oc")
                        w_in = wqkT.ap()[:, oc * 128:(oc + 1) * 128].rearrange(
                            "(c p) o -> p c o", p=128).bitcast(F32R)
                        if oc == 0:
                            with tc.high_priority():
                                nc.sync.dma_start(out=w_oc, in_=w_in)
                        else:
                            nc.sync.dma_start(out=w_oc, in_=w_in)
                        pk = pqk.tile([128, 2, 512], F32, tag="pk")
                        for hi, h in enumerate(H_ORDER):
                            for sc in range(2):
                                nc.tensor.matmul(
                                    pk[:, sc, :], w_oc[:, h, :],
                                    xt_slice(h, slice(sc * 512, (sc + 1) * 512)),
                                    start=(hi == 0), stop=(hi == 31))
                        # RoPE: q' = q*cos + shuffle16(q)*sin_signed
                        dst = qT_s if oc < 8 else kT_s
                        hh = oc % 8
                        for sc in range(2):
                            cs = cos_sb[:, sc * 512:(sc + 1) * 512]
                            sn = sin_sb[:, sc * 512:(sc + 1) * 512]
                            qrot = rpool.tile([128, 512], F32, tag="qrot")
                            nc.vector.stream_shuffle(qrot, pk[:, sc, :], SHUF_MASK)
                            t1 = rpool.tile([128, 512], F32, tag="t1")
                            nc.vector.tensor_tensor(t1, pk[:, sc, :], cs, ALU.mult)
                            t2 = rpool.tile([128, 512], F32, tag="t2")
                            nc.gpsimd.tensor_tensor(t2, qrot, sn, ALU.mult)
                            qk_o = spill.tile([128, 512], F32R, tag="qk_o")
                            nc.vector.tensor_tensor(qk_o, t1, t2, ALU.add)
                            nc.gpsimd.dma_start(
                                out=dst.ap()[hh * 128:(hh + 1) * 128,
                                             s0 + sc * 512: s0 + (sc + 1) * 512],
                                in_=qk_o)

                # --- v projection, x-stationary, out = v [s, o] --------------
                with ExitStack() as vv:
                    wvp = vv.enter_context(tc.tile_pool(name="wvt", bufs=4))
                    pv = vv.enter_context(
                        tc.tile_pool(name="pv", bufs=8, space="PSUM"))
                    vout = vv.enter_context(tc.tile_pool(name="vout", bufs=3))
                    for ov in range(2):          # v-dim chunks of 512
                        vb = [pv.tile([128, 512], F32, tag="vb", name=f"vb{i}")
                              for i in range(8)]
                        for hi, h in enumerate(H_ORDER):
                            wv_t = wvp.tile([128, 512], F32R, tag="wv_t")
                            nc.gpsimd.dma_start(
                                out=wv_t,
                                in_=wvT.ap()[h * 128:(h + 1) * 128,
                                             ov * 512:(ov + 1) * 512].bitcast(F32R))
                            for sc in range(8):
                                nc.tensor.matmul(
                                    vb[sc],
                                    xt_slice(h, slice(sc * 128, (sc + 1) * 128)),
                                    wv_t,
                                    start=(hi == 0), stop=(hi == 31))
                        for sc in range(8):
                            vo = vout.tile([128, 512], F32R, tag="vo")
                            nc.vector.tensor_copy(vo, vb[sc])
                            nc.gpsimd.dma_start(
                                out=v_s.ap()[s0 + sc * 128: s0 + (sc + 1) * 128,
                                             ov * 512:(ov + 1) * 512],
                                in_=vo)

        # ---------------- Phase B: attention, scores kept as S^T [k, q] ------
        def emit_attn(qcs, kspan):
            # kspan: number of 128-wide k blocks available (8 after half 0,
            # 16 after half 1); causal qcs<=1 only touch the first 8.
            with ExitStack() as phb:
                kvp = phb.enter_context(tc.tile_pool(name="kv", bufs=3))
                qp = phb.enter_context(tc.tile_pool(name="qtl", bufs=3))
                esp = phb.enter_context(tc.tile_pool(name="es", bufs=8))
                smallp = phb.enter_context(tc.tile_pool(name="small", bufs=3))
                ps = phb.enter_context(
                    tc.tile_pool(name="ps", bufs=3, space="PSUM"))
                pav = phb.enter_context(
                    tc.tile_pool(name="pav", bufs=2, space="PSUM"))
                pden = phb.enter_context(
                    tc.tile_pool(name="pden", bufs=2, space="PSUM"))
                pbc = phb.enter_context(
                    tc.tile_pool(name="pbc", bufs=1, space="PSUM"))
                mp = None
                if mask_mode == "full":
                    mp = phb.enter_context(tc.tile_pool(name="msk", bufs=3))
                if mask_mode == "causal":
                    mskp = phb.enter_context(tc.tile_pool(name="mskc", bufs=1))
                    mask_sb = mskp.tile([128, 4, 512], F32)
                    nc.sync.dma_start(
                        out=mask_sb, in_=masks.ap().rearrange("v p q -> p v q"))
                for hh in range(HEADS_PER_CORE):
                    ktile = kvp.tile([128, kspan * 128], F32R, tag="ktile",
                                     name=f"ktile{kspan}_{hh}")
                    nc.sync.dma_start(
                        out=ktile,
                        in_=kT_s.ap()[hh * 128:(hh + 1) * 128, 0:kspan * 128])
                    vtile = kvp.tile([128, kspan, 128], F32R, tag="vtile",
                                     name=f"vtile{kspan}_{hh}")
                    nc.sync.dma_start(
                        out=vtile,
                        in_=v_s.ap()[0:kspan * 128,
                                     hh * 128:(hh + 1) * 128].rearrange(
                            "(b p) d -> p b d", p=128))
                    for qc in qcs:
                        qtile = qp.tile([128, 512], F32R, tag="qtile")
                        nc.sync.dma_start(
                            out=qtile,
                            in_=qT_s.ap()[hh * 128:(hh + 1) * 128,
                                          qc * 512:(qc + 1) * 512])
                        nblk = 4 * qc + 4 if mask_mode == "causal" else 16
                        av = pav.tile([128, 512], F32, tag="av")
                        den = pden.tile([1, 512], F32, tag="den")
                        for kb in range(nblk):
                            # causal diagonal blocks only cover q >= 128v;
                            # trim N (min 256 for full-rate fp32r)
                            v = kb - 4 * qc
                            q_lo = 0
                            if mask_mode == "causal" and v > 0:
                                q_lo = min(128 * v, 256)
                            qs = slice(q_lo, 512)
                            sps = ps.tile([128, 512], F32, tag="sps")
                            nc.tensor.matmul(
                                sps[:, qs],
                                ktile[:, kb * 128:(kb + 1) * 128], qtile[:, qs],
                                start=True, stop=True)
                            if mask_mode == "causal" and v >= 0:
                                nc.vector.tensor_tensor(
                                    sps[:, qs], sps[:, qs],
                                    mask_sb[:, v, qs], ALU.add)
                            elif mask_mode == "full":
                                mt = mp.tile([128, 512], F32, tag="mt")
                                nc.sync.dma_start(
                                    out=mt,
                                    in_=maskT.ap()[kb * 128:(kb + 1) * 128,
                                                   qc * 512:(qc + 1) * 512])
                                nc.vector.tensor_tensor(sps, sps, mt, ALU.add)
                            es = esp.tile([128, 512], F32R, tag="es")
                            nc.scalar.activation(es[:, qs], sps[:, qs],
                                                 ACTF.Exp, scale=inv_sqrt_d)
                            nc.tensor.matmul(
                                av[:, qs], vtile[:, kb, :], es[:, qs],
                                start=(kb == 0), stop=(kb == nblk - 1))
                            nc.tensor.matmul(
                                den[:, qs], ones_col, es[:, qs],
                                start=(kb == 0), stop=(kb == nblk - 1))
                        recip_f = smallp.tile([1, 512], F32, tag="recip_f")
                        nc.vector.reciprocal_approx_fast(recip_f, den)
                        recip = smallp.tile([1, 512], F32R, tag="recip")
                        nc.vector.tensor_copy(recip, recip_f)
                        bc = pbc.tile([128, 512], F32, tag="bc")
                        nc.tensor.matmul(bc, ones_row, recip,
                                         start=True, stop=True)
                        bc_sb = smallp.tile([128, 512], F32, tag="bc_sb")
                        nc.vector.tensor_copy(bc_sb, bc)
                        at_t = smallp.tile([128, 512], F32R, tag="at_t")
                        nc.vector.tensor_tensor(at_t, av, bc_sb, ALU.mult)
                        nc.gpsimd.dma_start(
                            out=at_s.ap()[hh * 128:(hh + 1) * 128,
                                          qc * 512:(qc + 1) * 512],
                            in_=at_t)

        emit_proj_half(0)
        if mask_mode == "causal":
            emit_attn((1, 0), 8)       # q<1024 only needs k<1024 (half 0)
            emit_proj_half(1)
            emit_attn((3, 2), 16)
        else:
            emit_proj_half(1)
            emit_attn((3, 2, 1, 0), 16)

        # ---------------- Phase C: output projection -------------------------
        with ExitStack() as phc:
            atp = phc.enter_context(tc.tile_pool(name="atl", bufs=1))
            AT = atp.tile([128, HEADS_PER_CORE, S], F32R)
            for hc in range(HEADS_PER_CORE):
                nc.sync.dma_start(
                    out=AT[:, hc, :],
                    in_=at_s.ap()[hc * 128:(hc + 1) * 128, :])
            wop = phc.enter_context(tc.tile_pool(name="wo", bufs=2))
            pop = phc.enter_context(tc.tile_pool(name="pop", bufs=4, space="PSUM"))
            otp = phc.enter_context(tc.tile_pool(name="ot", bufs=3))
            for oc in range(8):                  # output chunks of 512
                wo_sl = wop.tile([128, 8, 512], F32R, tag="wo_sl")
                wo_in = woT.ap()[:, oc * 512:(oc + 1) * 512].rearrange(
                    "(c p) o -> p c o", p=128).bitcast(F32R)
                if oc == 0:
                    with tc.high_priority():
                        nc.sync.dma_start(out=wo_sl, in_=wo_in)
                else:
                    nc.sync.dma_start(out=wo_sl, in_=wo_in)
                for st in range(16):             # s tiles of 128
                    op = pop.tile([128, 512], F32, tag="op")
                    for hc in range(8):
                        nc.tensor.matmul(
                            op, AT[:, hc, st * 128:(st + 1) * 128],
                            wo_sl[:, hc, :],
                            start=(hc == 0), stop=(hc == 7))
                    ot = otp.tile([128, 512], F32, tag="ot")
                    nc.vector.tensor_copy(ot, op)
                    nc.sync.dma_start(
                        out=out_p.ap()[st * 128:(st + 1) * 128,
                                       oc * 512:(oc + 1) * 512],
                        in_=ot)

    nc.compile()
    return nc


_PROGRAM_CACHE = {}


def _get_program(mask_mode):
    if mask_mode not in _PROGRAM_CACHE:
        _PROGRAM_CACHE[mask_mode] = _build_program(mask_mode)
    return _PROGRAM_CACHE[mask_mode]


def _classify_mask(attention_mask):
    m = np.asarray(attention_mask)
    if not np.any(m):
        return "none"
    neg = np.float32(np.finfo(np.float32).min)
    causal = np.triu(np.full((S, S), neg, dtype=np.float32), k=1)
    for b in range(m.shape[0]):
        if not np.array_equal(m[b, 0], causal):
            return "full"
    return "causal"


def _prep_core_inputs(hidden_states, attention_mask, position_ids, W_pack, W_o,
                      mask_mode):
    hidden_states = np.ascontiguousarray(np.asarray(hidden_states, dtype=np.float32))
    W_pack = np.asarray(W_pack, dtype=np.float32)
    W_o = np.asarray(W_o, dtype=np.float32)
    pos = np.asarray(position_ids).astype(np.int64)

    cos_t, sin_t = _rope_tables(int(pos.max()) + 1)
    # per-batch gathered + transposed + row-permuted (+ sign folded into sin)
    cosT_b, sinT_b = [], []
    for b in range(B):
        c = cos_t[pos[b]][:, PERM].T.copy()              # [128, S]
        s = (sin_t[pos[b]][:, PERM] * SIGN[None, :]).T.copy()
        cosT_b.append(np.ascontiguousarray(c))
        sinT_b.append(np.ascontiguousarray(s))

    xT_b = [np.ascontiguousarray(hidden_states[b].T) for b in range(B)]

    mask4 = None
    maskT_b = None
    if mask_mode == "causal":
        kk = np.arange(128)[:, None]
        qq = np.arange(512)[None, :]
        mask4 = np.stack(
            [np.where(kk + 128 * v <= qq, 0.0, NEG_BIG).astype(np.float32)
             for v in range(4)])
    elif mask_mode == "full":
        m = np.asarray(attention_mask, dtype=np.float32)
        maskT_b = [np.ascontiguousarray(m[b, 0].T) for b in range(B)]

    in_maps = []
    for c in range(8):
        b, g = c // 4, c % 4
        # per-head d-permuted q/k weight rows, head-major columns in wqkT
        qrows = np.concatenate(
            [g * HG + hh * 128 + PERM for hh in range(HEADS_PER_CORE)])
        krows = HIDDEN + qrows
        vrows = 2 * HIDDEN + g * HG + np.arange(HG)
        wqkT = np.ascontiguousarray(
            np.concatenate([W_pack[qrows], W_pack[krows]], axis=0).T)
        wvT = np.ascontiguousarray(W_pack[vrows].T)
        woT = np.ascontiguousarray(W_o[:, g * HG:(g + 1) * HG].T)
        im = {"xT": xT_b[b], "wqkT": wqkT, "wvT": wvT, "woT": woT,
              "cosT": cosT_b[b], "sinT": sinT_b[b]}
        if mask_mode == "causal":
            im["masks"] = mask4
        elif mask_mode == "full":
            im["maskT"] = maskT_b[b]
        in_maps.append(im)
    return in_maps


def _run(hidden_states, attention_mask, position_ids, W_pack, W_o,
         trace=False, trace_kwargs=None):
    from concourse.bass_utils import run_bass_kernel_spmd

    mask_mode = _classify_mask(attention_mask)
    nc = _get_program(mask_mode)
    in_maps = _prep_core_inputs(hidden_states, attention_mask, position_ids,
                                W_pack, W_o, mask_mode)
    try:
        res = run_bass_kernel_spmd(nc, in_maps, list(range(8)), trace=trace,
                                   **(trace_kwargs or {}))
    except Exception:
        # transient NRT_EXEC_UNIT_UNRECOVERABLE wedges recover on retry
        import time as _time
        _time.sleep(15)
        res = run_bass_kernel_spmd(nc, in_maps, list(range(8)), trace=trace,
                                   **(trace_kwargs or {}))
    out = np.zeros((B, S, HIDDEN), dtype=np.float32)
    for c in range(8):
        out[c // 4] += res.results[c]["out_p"]
    return out, res


def kernel(hidden_states, attention_mask, position_ids, W_pack, W_o):
    out, _ = _run(hidden_states, attention_mask, position_ids, W_pack, W_o)
    return out
